# revision 1
# baseline (speedup 1.0000x reference)
"""DiceLoss (CondInst-style dynamic mask head) Trainium2 kernel.

Data-parallel over batch B=8: one image per NeuronCore. Per image:
  - gather per-object 1x1-conv weights from conv_weight at ind (host, tiny)
  - conv1: [10 -> 8] per object (relu), conv2: [8 -> 8] (relu),
    conv3: [8 -> 1] (sigmoid) over all HW=16384 pixels, K=32 objects
  - dice partial sums sum(p*t), sum(p*p) per image (sum(t*t) is
    pred-independent and computed on host)

Host folds the relative-coordinate channels into conv1's bias (they are
affine in the pixel grid), builds block-diagonal weights so all 32 objects
run as dense 128-contraction matmuls, pre-masks target, and forces
sigmoid->0 for masked objects via a large negative conv3 bias.

Device layout (per core), processed in 8 groups of 4 hw-chunks (512 px):
  conv1: two 16-object halves row-packed via tile_position (0,0)/(32,0),
         so both halves stream through the PE concurrently; 2-bank-wide
         PSUM tiles halve the evacuation op count.
  conv2: weight-batched (all 4 chunks of half A, then half B).
  conv3: col-tiled - chunk j lands at PSUM partitions 32j, so 4 chunks
         pack one bank and sigmoid runs on full 128-partition tiles.
  dice:  pt/pp products on DVE, summed over hw by PE ones-matmuls into a
         single shared PSUM bank (cols 0:256 pt, 256:512 pp).
"""

import numpy as np
import ml_dtypes

import concourse.bass as bass
import concourse.mybir as mybir
import concourse.tile as tile
from concourse.bass_utils import run_bass_kernel_spmd

BF16 = mybir.dt.bfloat16
F32 = mybir.dt.float32

B, C, K, H, W = 8, 8, 32, 128, 128
HW = H * W
CW = 169
CHUNK = 512
NGROUP = 8           # groups of 4 chunks
N_CORES = 8

_NEG_BIG = 30000.0   # sigmoid(z - 30000) == 0 for any realistic z


# ---------------------------------------------------------------------------
# Workarounds for this walrus build's 1-sem-wait-per-instruction encoding
# limit: split Tile's multi-wait drain and spill excess waits onto NoOps.
# ---------------------------------------------------------------------------
def _drain_and_barrier_split(self, tick_clock, wait_clock):
    from concourse.tile import ScopedClock

    nc = self.nc
    drain_inst = nc.sync.drain()
    wait_clock.add_sem_waits(
        drain_inst.ins, ScopedClock({None: tick_clock.global_clock})
    )
    si = drain_inst.ins.sync_info
    waits = list(si.on_wait) if si is not None else []
    if len(waits) > 1:
        drain_inst.ins.sync_info = None
        handles = list(self.sems.allocated().values())
        by_num = {h.num: h for h in handles}
        by_name = {h.name: h for h in handles}
        for w_ in waits:
            h = by_num.get(w_.id) or by_name.get(w_.ant_name)
            assert h is not None, f"no semaphore handle for {w_}"
            assert w_.wait_mode == "sem-ge-imm", w_.wait_mode
            nc.sync.wait_ge(h, w_.wait_value)
    nc.all_engine_barrier()
    popped = nc._tile_sem_poison_stack.pop()
    assert popped is self._sem_poison
    nc.clear_and_free_semaphores(list(self.sems.allocated().values()))
    nc.all_engine_barrier()


tile.TileContext._drain_and_barrier = _drain_and_barrier_split


def split_excess_waits(nc, register=True):
    for f in nc.m.functions:
        for bb in f.blocks:
            out = []
            changed = False
            for inst in bb.instructions:
                si = inst.sync_info
                waits = list(si.on_wait) if si is not None else []
                if len(waits) > 1:
                    keep, spill = waits[:1], waits[1:]
                    for i, w_ in enumerate(spill):
                        nop = mybir.InstNoOp(
                            name=f"{inst.name}_wspill{i}",
                            engine=inst.engine,
                            sync_info=mybir.SyncInfo(on_wait=[w_], on_update=[]),
                            bass_nofuse=True,
                        )
                        if register:
                            nc.register_instruction(nop, overwrite=True)
                        out.append(nop)
                    inst.sync_info = mybir.SyncInfo(
                        on_wait=keep, on_update=list(si.on_update)
                    )
                    changed = True
                out.append(inst)
            if changed:
                bb.instructions = out


# ---------------------------------------------------------------------------
# Device kernel
# ---------------------------------------------------------------------------
def build_nc():
    nc = bass.Bass()
    f10_d = nc.declare_dram_parameter("f10", [10, HW], BF16, False)
    w1t_d = nc.declare_dram_parameter("w1t", [42, 128], BF16, False)
    w2t_d = nc.declare_dram_parameter("w2t", [128, 256], BF16, False)
    w3t_d = nc.declare_dram_parameter("w3t", [128, 64], BF16, False)
    b12_d = nc.declare_dram_parameter("b12", [128, 4], F32, False)
    b3_d = nc.declare_dram_parameter("b3", [128, 1], F32, False)
    tpk_d = nc.declare_dram_parameter("tpk", [128, 4096], BF16, False)
    red_d = nc.declare_dram_parameter("red", [1, 512], F32, True)

    RELU = mybir.ActivationFunctionType.Relu
    SIGM = mybir.ActivationFunctionType.Sigmoid
    ADD = mybir.AluOpType.add
    MAX = mybir.AluOpType.max

    with tile.TileContext(nc) as tc:
        with (
            tc.tile_pool(name="const", bufs=1) as const,
            tc.tile_pool(name="h1p", bufs=2) as h1p,
            tc.tile_pool(name="h2p", bufs=5) as h2p,
            tc.tile_pool(name="predp", bufs=2) as predp,
            tc.tile_pool(name="prodp", bufs=2) as prodp,
            tc.tile_pool(name="ps1ap", bufs=1, space="PSUM") as ps1ap,
            tc.tile_pool(name="ps1bp", bufs=1, space="PSUM") as ps1bp,
            tc.tile_pool(name="ps2p", bufs=1, space="PSUM") as ps2p,
            tc.tile_pool(name="ps3p", bufs=1, space="PSUM") as ps3p,
            tc.tile_pool(name="psredp", bufs=1, space="PSUM") as psredp,
        ):
            w1_sb = const.tile([42, 128], BF16)
            nc.gpsimd.dma_start(out=w1_sb[:], in_=w1t_d[:])
            w2_sb = const.tile([128, 256], BF16)
            nc.gpsimd.dma_start(out=w2_sb[:], in_=w2t_d[:])
            w3_sb = const.tile([128, 64], BF16)
            nc.gpsimd.dma_start(out=w3_sb[:], in_=w3t_d[:])
            b12_sb = const.tile([128, 4], F32)
            nc.gpsimd.dma_start(out=b12_sb[:], in_=b12_d[:])
            b3_sb = const.tile([128, 1], F32)
            nc.gpsimd.dma_start(out=b3_sb[:], in_=b3_d[:])
            f_sb = const.tile([42, HW], BF16)
            nc.gpsimd.dma_start(out=f_sb[0:10, :], in_=f10_d[:])
            nc.sync.dma_start(out=f_sb[32:42, :], in_=f10_d[:])
            tpk_sb = const.tile([128, 4096], BF16)
            nc.gpsimd.dma_start(out=tpk_sb[:], in_=tpk_d[:])

            ones_sb = const.tile([128, 1], BF16)
            nc.vector.memset(ones_sb, 1.0)
            # shared accumulator bank: cols 0:256 pt, 256:512 pp
            red = psredp.tile([1, 512], F32)

            def evac_relu(dst, src, bias_ap, on_act):
                if on_act:
                    nc.scalar.activation(
                        out=dst, in_=src, func=RELU, bias=bias_ap
                    )
                else:
                    nc.vector.tensor_scalar(
                        out=dst, in0=src, scalar1=bias_ap, scalar2=0.0,
                        op0=ADD, op1=MAX,
                    )

            first_red = [True]

            def reduce_into(cols, prod):
                # red[0, cols] += column sums of prod (two N=256 matmuls)
                for h_ in range(2):
                    nc.tensor.matmul(
                        red[0:1, cols],
                        ones_sb[:],
                        prod[:, bass.ts(h_, 256)],
                        start=first_red[0],
                        stop=False,
                        skip_group_check=True,
                    )
                    first_red[0] = False

            for g in range(NGROUP):
                c0 = 4 * g
                # ---- conv1: row-packed halves, 2 chunk-pairs -> wide psum
                ps1a = [None, None]
                ps1b = [None, None]
                h1a = [None, None]
                h1b = [None, None]
                for p in range(2):
                    ps1a[p] = ps1ap.tile([128, 1024], F32, tag="ps1a", name="ps1a")
                    ps1b[p] = ps1bp.tile([128, 1024], F32, tag="ps1b", name="ps1b")
                    for i in range(2):
                        cs = bass.ts(c0 + 2 * p + i, CHUNK)
                        ncol = bass.ts(i, CHUNK)
                        nc.tensor.matmul(
                            ps1a[p][:, ncol], w1_sb[0:10, :], f_sb[0:10, cs],
                            start=True, stop=True, tile_position=(0, 0),
                        )
                        nc.tensor.matmul(
                            ps1b[p][:, ncol], w1_sb[32:42, :], f_sb[32:42, cs],
                            start=True, stop=True, tile_position=(32, 0),
                        )
                    h1a[p] = h1p.tile([128, 1024], BF16, tag="h1a", name="h1a")
                    evac_relu(h1a[p][:], ps1a[p][:], b12_sb[:, 0:1],
                              on_act=(p == 1))
                    h1b[p] = h1p.tile([128, 1024], BF16, tag="h1b", name="h1b")
                    evac_relu(h1b[p][:], ps1b[p][:], b12_sb[:, 1:2],
                              on_act=(p == 0))

                # ---- conv2: weight-batched (A over 4 chunks, then B)
                h2a = [None] * 4
                h2b = [None] * 4
                for p in range(2):
                    ps2 = ps2p.tile([128, 1024], F32, tag="ps2")
                    for i in range(2):
                        j = 2 * p + i
                        nc.tensor.matmul(
                            ps2[:, bass.ts(i, CHUNK)], w2_sb[:, 0:128],
                            h1a[p][:, bass.ts(i, CHUNK)],
                            start=True, stop=True,
                        )
                    for i in range(2):
                        j = 2 * p + i
                        h2a[j] = h2p.tile([128, CHUNK], BF16, tag="h2a", name="h2a")
                        evac_relu(h2a[j][:], ps2[:, bass.ts(i, CHUNK)],
                                  b12_sb[:, 2:3], on_act=(j % 2 == 0))
                for p in range(2):
                    ps2 = ps2p.tile([128, 1024], F32, tag="ps2")
                    for i in range(2):
                        j = 2 * p + i
                        nc.tensor.matmul(
                            ps2[:, bass.ts(i, CHUNK)], w2_sb[:, 128:256],
                            h1b[p][:, bass.ts(i, CHUNK)],
                            start=True, stop=True,
                        )
                    for i in range(2):
                        j = 2 * p + i
                        h2b[j] = h2p.tile([128, CHUNK], BF16, tag="h2b", name="h2b")
                        evac_relu(h2b[j][:], ps2[:, bass.ts(i, CHUNK)],
                                  b12_sb[:, 3:4], on_act=(j % 2 == 1))

                # ---- conv3: batched A then B, col-tiled into one bank
                ps3 = ps3p.tile([128, CHUNK], F32, tag="ps3")
                for j in range(4):
                    nc.tensor.matmul(
                        ps3[32 * j : 32 * j + 32, :], w3_sb[:, 0:32],
                        h2a[j][:],
                        start=True, stop=False,
                        tile_position=(0, 32 * j),
                        skip_group_check=True,
                    )
                for j in range(4):
                    nc.tensor.matmul(
                        ps3[32 * j : 32 * j + 32, :], w3_sb[:, 32:64],
                        h2b[j][:],
                        start=False, stop=True,
                        tile_position=(0, 32 * j),
                        skip_group_check=True,
                    )

                # ---- sigmoid + dice products
                pred = predp.tile([128, CHUNK], BF16, tag="pred")
                nc.scalar.activation(
                    out=pred[:], in_=ps3[:], func=SIGM, bias=b3_sb[:, 0:1]
                )
                tgt = tpk_sb[:, bass.ts(g, CHUNK)]
                pt_s = prodp.tile([128, CHUNK], BF16, tag="pt_s")
                nc.vector.tensor_mul(out=pt_s[:], in0=pred[:], in1=tgt)
                reduce_into(slice(0, 256), pt_s)
                pp_s = prodp.tile([128, CHUNK], BF16, tag="pp_s")
                nc.vector.tensor_mul(out=pp_s[:], in0=pred[:], in1=pred[:])
                reduce_into(slice(256, 512), pp_s)

            red_sb = const.tile([1, 512], F32)
            nc.scalar.copy(out=red_sb[:], in_=red[:])
            nc.gpsimd.dma_start(out=red_d[:], in_=red_sb[:])
    split_excess_waits(nc)
    return nc


# ---------------------------------------------------------------------------
# Host-side input preparation (numpy, per image)
# ---------------------------------------------------------------------------
def prep_inputs(seg_feat, conv_weight, mask, ind, target):
    seg_feat = np.asarray(seg_feat)
    conv_weight = np.asarray(conv_weight)
    mask = np.asarray(mask)
    ind = np.asarray(ind).astype(np.int64)
    target = np.asarray(target)

    cw = conv_weight.reshape(B, CW, HW)
    w = np.take_along_axis(cw, ind[:, None, :], axis=2)  # [B, CW, K]
    w = np.ascontiguousarray(w.transpose(0, 2, 1)).astype(np.float32)  # [B,K,CW]

    c1w = w[..., 0:80].reshape(B, K, C, C + 2)
    c1b = w[..., 80:88]
    c2w = w[..., 88:152].reshape(B, K, C, C)
    c2b = w[..., 152:160]
    c3w = w[..., 160:168].reshape(B, K, C)
    c3b = w[..., 168]

    x = (ind % W).astype(np.float32) / W
    y = (ind // W).astype(np.float32) / H
    b1eff = c1b - c1w[..., 8] * x[:, :, None] - c1w[..., 9] * y[:, :, None]

    mf = mask.astype(np.float32)
    b3eff = c3b - _NEG_BIG * (1.0 - mf)

    xg = (np.arange(HW, dtype=np.float32) % W) / W
    yg = (np.arange(HW, dtype=np.float32) // W) / H

    bf = ml_dtypes.bfloat16
    in_maps = []
    tt_host = np.empty(B, np.float64)
    for b in range(B):
        f10 = np.concatenate(
            [seg_feat[b].reshape(C, HW), xg[None], yg[None]], axis=0
        ).astype(bf)

        w1half = c1w[b].transpose(2, 0, 1).reshape(C + 2, K * C)  # [10, 256]
        w1t = np.zeros((42, 128), np.float32)
        w1t[0:10, :] = w1half[:, 0:128]
        w1t[32:42, :] = w1half[:, 128:256]
        w1t = w1t.astype(bf)

        w2t = np.zeros((128, 256), np.float32)
        for half in range(2):
            for kl in range(16):
                blk = c2w[b, half * 16 + kl].T  # [c, o]
                w2t[kl * 8 : kl * 8 + 8,
                    half * 128 + kl * 8 : half * 128 + kl * 8 + 8] = blk
        w2t = w2t.astype(bf)

        w3t = np.zeros((128, 64), np.float32)
        for half in range(2):
            for kl in range(16):
                kk = half * 16 + kl
                w3t[kl * 8 : kl * 8 + 8, half * 32 + kk] = c3w[b, kk]
        w3t = w3t.astype(bf)

        b12 = np.stack(
            [
                b1eff[b].reshape(K * C)[0:128],
                b1eff[b].reshape(K * C)[128:256],
                c2b[b].reshape(K * C)[0:128],
                c2b[b].reshape(K * C)[128:256],
            ],
            axis=1,
        ).astype(np.float32)

        b3 = np.tile(b3eff[b], 4)[:, None].astype(np.float32)

        t_m = (target[b] * mf[b][:, None, None]).reshape(K, HW)
        tt_host[b] = np.square(t_m, dtype=np.float64).sum()
        tpk = np.ascontiguousarray(
            t_m.reshape(K, 8, 4, CHUNK).transpose(2, 0, 1, 3).reshape(128, 4096)
        ).astype(bf)

        in_maps.append(
            {
                "f10": f10,
                "w1t": w1t,
                "w2t": w2t,
                "w3t": w3t,
                "b12": b12,
                "b3": b3,
                "tpk": tpk,
            }
        )
    return in_maps, tt_host


def finish(red_list, tt_host):
    per_img = np.empty(B, np.float64)
    for b in range(B):
        r = np.asarray(red_list[b], np.float64)  # [1, 512]
        inter = r[0, 0:256].sum()
        spp = r[0, 256:512].sum()
        stt = tt_host[b]
        per_img[b] = 1.0 - (2.0 * inter + 1.0) / (spp + stt + 1.0)
    return np.float32(per_img.mean())


_NC_CACHE = {}


def kernel(seg_feat, conv_weight, mask, ind, target):
    if "nc" not in _NC_CACHE:
        _NC_CACHE["nc"] = build_nc()
    nc = _NC_CACHE["nc"]
    in_maps, tt_host = prep_inputs(seg_feat, conv_weight, mask, ind, target)
    res = run_bass_kernel_spmd(nc, in_maps, list(range(N_CORES)))
    return finish([res.results[b]["red"] for b in range(B)], tt_host)



# revision 2
# speedup vs baseline: 1.0005x; 1.0005x over previous
"""DiceLoss (CondInst-style dynamic mask head) Trainium2 kernel, v2: fp8 DoubleRow.

Data-parallel over batch B=8: one image per NeuronCore. All three per-object
1x1 convs run as fp8e4 DoubleRow matmuls (0.5 PE cycles per output column,
2x the bf16 column rate):
  conv1 [10->8]:  true channel-pairing; host lays out features as [5,2,HW]
                  (channel pairs in the plane dim), one matmul per 512-px
                  chunk per 16-object half -> PSUM [128,512] in 256 cycles.
  conv2 [8->8]:   zero-padded plane pairing; weights duplicated as (W,0) and
                  (0,W) planes so the two matmuls of a chunk-pair read one
                  natural [128,2,512] h1 tile and write its two chunks.
  conv3 [8->1]:   both-planes trick; lhsT [128,2,32] carries W3 in plane 0
                  cols 0:16 and plane 1 cols 16:32, so ONE matmul emits both
                  chunks of a pair (16 objects each) into 32 PSUM partitions.
Dice reductions run off the PE: DVE computes pred*tgt, Pool (gpsimd)
tensor-reduces it to a scalar, ACT squares pred with accum_out. Host folds
relative-coordinate channels into conv1's bias, pre-masks target, forces
sigmoid->0 for masked objects via a large negative conv3 bias, and computes
sum(t*t).

PSUM: one rotating pool of [128,2,512] tiles (2 banks x 3 bufs = 6 banks)
for conv1/conv2, plus [128,512] x 2 for conv3. Evacuations (relu+bias+fp8
quantize) rotate across DVE/Pool/ACT.
"""

import numpy as np
import ml_dtypes

import concourse.bass as bass
import concourse.mybir as mybir
import concourse.tile as tile
from concourse.bass_utils import run_bass_kernel_spmd

FP8 = mybir.dt.float8e4
BF16 = mybir.dt.bfloat16
F32 = mybir.dt.float32
DR = mybir.MatmulPerfMode.DoubleRow

B, C, K, H, W = 8, 8, 32, 128, 128
HW = H * W
CW = 169
N_CORES = 8
NPAIR = 16           # pairs of 512-px chunks

_NEG_BIG = 30000.0


# ---------------------------------------------------------------------------
# Workarounds for this walrus build's 1-sem-wait-per-instruction encoding
# limit: split Tile's multi-wait drain and spill excess waits onto NoOps.
# ---------------------------------------------------------------------------
def _drain_and_barrier_split(self, tick_clock, wait_clock):
    from concourse.tile import ScopedClock

    nc = self.nc
    drain_inst = nc.sync.drain()
    wait_clock.add_sem_waits(
        drain_inst.ins, ScopedClock({None: tick_clock.global_clock})
    )
    si = drain_inst.ins.sync_info
    waits = list(si.on_wait) if si is not None else []
    if len(waits) > 1:
        drain_inst.ins.sync_info = None
        handles = list(self.sems.allocated().values())
        by_num = {h.num: h for h in handles}
        by_name = {h.name: h for h in handles}
        for w_ in waits:
            h = by_num.get(w_.id) or by_name.get(w_.ant_name)
            assert h is not None, f"no semaphore handle for {w_}"
            assert w_.wait_mode == "sem-ge-imm", w_.wait_mode
            nc.sync.wait_ge(h, w_.wait_value)
    nc.all_engine_barrier()
    popped = nc._tile_sem_poison_stack.pop()
    assert popped is self._sem_poison
    nc.clear_and_free_semaphores(list(self.sems.allocated().values()))
    nc.all_engine_barrier()


tile.TileContext._drain_and_barrier = _drain_and_barrier_split


def split_excess_waits(nc, register=True):
    for f in nc.m.functions:
        for bb in f.blocks:
            out = []
            changed = False
            for inst in bb.instructions:
                si = inst.sync_info
                waits = list(si.on_wait) if si is not None else []
                if len(waits) > 1:
                    keep, spill = waits[:1], waits[1:]
                    for i, w_ in enumerate(spill):
                        nop = mybir.InstNoOp(
                            name=f"{inst.name}_wspill{i}",
                            engine=inst.engine,
                            sync_info=mybir.SyncInfo(on_wait=[w_], on_update=[]),
                            bass_nofuse=True,
                        )
                        if register:
                            nc.register_instruction(nop, overwrite=True)
                        out.append(nop)
                    inst.sync_info = mybir.SyncInfo(
                        on_wait=keep, on_update=list(si.on_update)
                    )
                    changed = True
                out.append(inst)
            if changed:
                bb.instructions = out


# ---------------------------------------------------------------------------
# Device kernel
# ---------------------------------------------------------------------------
def build_nc():
    nc = bass.Bass()
    f_d = [
        nc.declare_dram_parameter(f"f{j}", [5, 2, 4096], FP8, False)
        for j in range(4)
    ]
    w1_d = nc.declare_dram_parameter("w1", [5, 2, 256], FP8, False)
    w2_d = nc.declare_dram_parameter("w2", [128, 2, 512], FP8, False)
    w3_d = nc.declare_dram_parameter("w3", [128, 2, 64], FP8, False)
    b12_d = nc.declare_dram_parameter("b12", [128, 4], F32, False)
    b3_d = nc.declare_dram_parameter("b3", [128, 1], F32, False)
    tpk_d = [
        nc.declare_dram_parameter(f"tpk{j}", [128, 2048], BF16, False)
        for j in range(2)
    ]
    red_d = nc.declare_dram_parameter("red", [128, 16], F32, True)
    dbg_d = nc.declare_dram_parameter("dbg", [1, 512], BF16, True)

    RELU = mybir.ActivationFunctionType.Relu
    SIGM = mybir.ActivationFunctionType.Sigmoid
    SQ = mybir.ActivationFunctionType.Square
    ADD = mybir.AluOpType.add
    MAX = mybir.AluOpType.max
    XYZWC = mybir.AxisListType.XYZWC

    with tile.TileContext(nc) as tc:
        with (
            tc.tile_pool(name="const", bufs=1) as const,
            tc.tile_pool(name="h1p", bufs=3) as h1p,
            tc.tile_pool(name="h2p", bufs=3) as h2p,
            tc.tile_pool(name="predp", bufs=2) as predp,
            tc.tile_pool(name="prodp", bufs=2) as prodp,
            tc.tile_pool(name="ps1p", bufs=2, space="PSUM") as ps1p,
            tc.tile_pool(name="ps2p", bufs=2, space="PSUM") as ps2p,
            tc.tile_pool(name="ps3p", bufs=2, space="PSUM") as ps3p,
        ):
            w1_sb = const.tile([5, 2, 256], FP8)
            nc.gpsimd.dma_start(out=w1_sb[:], in_=w1_d[:])
            b12_sb = const.tile([128, 4], F32)
            nc.gpsimd.dma_start(out=b12_sb[:], in_=b12_d[:])
            b3_sb = const.tile([128, 1], F32)
            nc.gpsimd.dma_start(out=b3_sb[:], in_=b3_d[:])
            w2_sb = const.tile([128, 2, 512], FP8)
            nc.gpsimd.dma_start(out=w2_sb[:], in_=w2_d[:])
            w3_sb = const.tile([128, 2, 64], FP8)
            nc.gpsimd.dma_start(out=w3_sb[:], in_=w3_d[:])
            f_sb = []
            for j in range(4):
                t = const.tile([5, 2, 4096], FP8, name=f"f{j}")
                nc.sync.dma_start(out=t[:], in_=f_d[j][:])
                f_sb.append(t)
            tpk_sb = []
            for j in range(2):
                t = const.tile([128, 2048], BF16, name=f"tpk{j}")
                nc.gpsimd.dma_start(out=t[:], in_=tpk_d[j][:])
                tpk_sb.append(t)

            red_sb = const.tile([128, 16], F32)
            junk = const.tile([128, 512], BF16)
            # the bass preamble memsets these const tiles unconditionally;
            # this verifier build rejects never-read memory locations, so
            # give each a reader (junk is DMA'd out via dbg).
            for ci, key in enumerate([(F32, 1.0), (BF16, 1.0),
                                      (mybir.dt.uint8, 127)]):
                nc.vector.tensor_copy(out=junk[:, ci: ci + 1],
                                      in_=nc.const_aps.aps[key])

            def evac(eng, dst, src, bias_ap):
                if eng == 0:
                    nc.scalar.activation(out=dst, in_=src, func=RELU,
                                         bias=bias_ap)
                elif eng == 1:
                    nc.vector.tensor_scalar(out=dst, in0=src,
                                            scalar1=bias_ap, scalar2=0.0,
                                            op0=ADD, op1=MAX)
                else:
                    nc.gpsimd.tensor_scalar(out=dst, in0=src,
                                            scalar1=bias_ap, scalar2=0.0,
                                            op0=ADD, op1=MAX)

            # engine rotation per pair parity:
            # [e1Ac, e1Ac1, e1Bc, e1Bc1, e2A, e2B]; 0=ACT 1=DVE 2=Pool.
            # ACT runs all sigmoids + square + pt-reduce, Pool the product.
            # Pool can't evac from PSUM to fp8, but bf16 SBUF ops are fine.
            ROT = [[1, 0, 1, 0, 1, 0], [1, 0, 1, 1, 1, 0]]

            ps3 = None
            for p in range(NPAIR):
                g, lp = p // 2, p % 2
                fj = f_sb[p // 4]
                off = (p % 4) * 1024
                rot = ROT[lp]

                h1a = h1p.tile([128, 2, 512], FP8, tag="h1a", name="h1a")
                h1b = h1p.tile([128, 2, 512], FP8, tag="h1b", name="h1b")
                for half, (h1t, wof, bof) in enumerate(
                    [(h1a, 0, 0), (h1b, 128, 1)]
                ):
                    for cc in range(2):
                        ps1 = ps1p.tile([128, 512], F32, tag="ps1", name="ps1")
                        nc.tensor.matmul(
                            ps1[:], w1_sb[:, :, wof: wof + 128],
                            fj[:, :, off + cc * 512: off + cc * 512 + 512],
                            start=True, stop=True, perf_mode=DR,
                        )
                        evac(rot[2 * half + cc], h1t[:, cc, :], ps1[:],
                             b12_sb[:, bof: bof + 1])

                ps2a = ps2p.tile([128, 2, 512], F32, tag="ps2", name="ps2a")
                for pl in range(2):
                    nc.tensor.matmul(
                        ps2a[:, pl, :], w2_sb[:, :, pl * 128: pl * 128 + 128],
                        h1a[:], start=True, stop=True, perf_mode=DR,
                    )
                h2a = h2p.tile([128, 2, 512], FP8, tag="h2a", name="h2a")
                evac(rot[4], h2a[:], ps2a[:], b12_sb[:, 2:3])

                ps2b = ps2p.tile([128, 2, 512], F32, tag="ps2", name="ps2b")
                for pl in range(2):
                    nc.tensor.matmul(
                        ps2b[:, pl, :], w2_sb[:, :, 256 + pl * 128: 256 + pl * 128 + 128],
                        h1b[:], start=True, stop=True, perf_mode=DR,
                    )
                h2b = h2p.tile([128, 2, 512], FP8, tag="h2b", name="h2b")
                evac(rot[5], h2b[:], ps2b[:], b12_sb[:, 3:4])

                if lp == 0:
                    pred = predp.tile([128, 512], BF16, tag="pred")
                for half, h2t in enumerate([h2a, h2b]):
                    ps3 = ps3p.tile([32, 512], F32, tag="ps3", name="ps3")
                    nc.tensor.matmul(
                        ps3[:], w3_sb[:, :, 32 * half: 32 * half + 32],
                        h2t[:], start=True, stop=True, perf_mode=DR,
                    )
                    qof = 64 * lp + 32 * half
                    nc.scalar.activation(
                        out=pred[qof: qof + 32, :], in_=ps3[:], func=SIGM,
                        bias=b3_sb[qof: qof + 32, 0:1],
                    )

                if lp == 1:
                    prod = prodp.tile([128, 512], BF16, tag="prod")
                    nc.gpsimd.tensor_mul(
                        out=prod[:], in0=pred[:],
                        in1=tpk_sb[g // 4][:, (g % 4) * 512: (g % 4) * 512 + 512],
                    )
                    nc.vector.tensor_reduce(
                        out=red_sb[:, 8 + g: 9 + g], in_=prod[:],
                        axis=mybir.AxisListType.X, op=ADD,
                    )
                    prod2 = prodp.tile([128, 512], BF16, tag="prod2")
                    nc.gpsimd.tensor_mul(out=prod2[:], in0=pred[:],
                                         in1=pred[:])
                    nc.vector.tensor_reduce(
                        out=red_sb[:, g: g + 1], in_=prod2[:],
                        axis=mybir.AxisListType.X, op=ADD,
                    )

            nc.gpsimd.dma_start(out=red_d[:], in_=red_sb[:])
            nc.gpsimd.dma_start(out=dbg_d[:], in_=junk[0:1, :])
    split_excess_waits(nc)
    return nc


# ---------------------------------------------------------------------------
# Host-side input preparation (numpy, per image)
# ---------------------------------------------------------------------------
def prep_inputs(seg_feat, conv_weight, mask, ind, target):
    seg_feat = np.asarray(seg_feat)
    conv_weight = np.asarray(conv_weight)
    mask = np.asarray(mask)
    ind = np.asarray(ind).astype(np.int64)
    target = np.asarray(target)

    cw = conv_weight.reshape(B, CW, HW)
    w = np.take_along_axis(cw, ind[:, None, :], axis=2)  # [B, CW, K]
    w = np.ascontiguousarray(w.transpose(0, 2, 1)).astype(np.float32)  # [B,K,CW]

    c1w = w[..., 0:80].reshape(B, K, C, C + 2)
    c1b = w[..., 80:88]
    c2w = w[..., 88:152].reshape(B, K, C, C)
    c2b = w[..., 152:160]
    c3w = w[..., 160:168].reshape(B, K, C)
    c3b = w[..., 168]

    x = (ind % W).astype(np.float32) / W
    y = (ind // W).astype(np.float32) / H
    b1eff = c1b - c1w[..., 8] * x[:, :, None] - c1w[..., 9] * y[:, :, None]

    mf = mask.astype(np.float32)
    b3eff = c3b - _NEG_BIG * (1.0 - mf)

    xg = (np.arange(HW, dtype=np.float32) % W) / W
    yg = (np.arange(HW, dtype=np.float32) // W) / H

    f8 = ml_dtypes.float8_e4m3
    bf = ml_dtypes.bfloat16

    # conv3/pred partition layout: q = 64*lp + 32*half + 16*cc + ko
    q = np.arange(128)
    q_half = (q // 32) % 2
    q_obj = 16 * q_half + (q % 16)      # [128] object id
    q_lp = q // 64
    q_cc = (q // 16) % 2

    in_maps = []
    tt_host = np.empty(B, np.float64)
    for b in range(B):
        f10 = np.concatenate(
            [seg_feat[b].reshape(C, HW), xg[None], yg[None]], axis=0
        )
        f_dr = f10.reshape(5, 2, HW).astype(f8)

        # conv1 weights: [ic, o*8+oc] -> [5, 2, 128] per half
        w1 = np.zeros((5, 2, 256), np.float32)
        for half in range(2):
            tmp = c1w[b, 16 * half: 16 * half + 16, :, 0:10]  # [16, 8, 10]
            w1[:, :, 128 * half: 128 * half + 128] = (
                tmp.transpose(2, 0, 1).reshape(5, 2, 128)
            )
        w1 = w1.astype(f8)

        # conv2: block-diagonal [128,128] per half, planes (W,0)/(0,W)
        w2 = np.zeros((128, 2, 512), np.float32)
        for half in range(2):
            W2 = np.zeros((128, 128), np.float32)
            for kl in range(16):
                W2[kl * 8: kl * 8 + 8, kl * 8: kl * 8 + 8] = \
                    c2w[b, 16 * half + kl].T
            w2[:, 0, 256 * half: 256 * half + 128] = W2
            w2[:, 1, 256 * half + 128: 256 * half + 256] = W2
        w2 = w2.astype(f8)

        # conv3: [128, 2, 32] per half; plane0 cols0:16, plane1 cols16:32
        w3 = np.zeros((128, 2, 64), np.float32)
        for half in range(2):
            W3 = np.zeros((128, 16), np.float32)
            for kl in range(16):
                W3[kl * 8: kl * 8 + 8, kl] = c3w[b, 16 * half + kl]
            w3[:, 0, 32 * half: 32 * half + 16] = W3
            w3[:, 1, 32 * half + 16: 32 * half + 32] = W3
        w3 = w3.astype(f8)

        b12 = np.stack(
            [
                b1eff[b, 0:16].reshape(128),
                b1eff[b, 16:32].reshape(128),
                c2b[b, 0:16].reshape(128),
                c2b[b, 16:32].reshape(128),
            ],
            axis=1,
        ).astype(np.float32)

        b3 = b3eff[b][q_obj][:, None].astype(np.float32)

        t_m = (target[b] * mf[b][:, None, None]).reshape(K, HW)
        tt_host[b] = np.square(t_m, dtype=np.float64).sum()
        # tpk[q, g*512 + px] = t_m[q_obj, (4g + 2*q_lp + q_cc)*512 + px]
        t_chunks = t_m.reshape(K, 32, 512)
        tpk = np.empty((128, 8, 512), np.float32)
        for g in range(8):
            cidx = 4 * g + 2 * q_lp + q_cc  # [128]
            tpk[:, g, :] = t_chunks[q_obj, cidx, :]
        tpk = tpk.reshape(128, 4096).astype(bf)

        im = {
            "w1": w1, "w2": w2, "w3": w3, "b12": b12, "b3": b3,
            "tpk0": tpk[:, 0:2048], "tpk1": tpk[:, 2048:4096],
        }
        for j in range(4):
            im[f"f{j}"] = np.ascontiguousarray(
                f_dr[:, :, j * 4096: j * 4096 + 4096]
            )
        in_maps.append(im)
    return in_maps, tt_host


def finish(red_list, tt_host):
    per_img = np.empty(B, np.float64)
    for b in range(B):
        r = np.asarray(red_list[b], np.float64)  # [128, 16]
        inter = r[:, 8:16].sum()
        spp = r[:, 0:8].sum()
        stt = tt_host[b]
        per_img[b] = 1.0 - (2.0 * inter + 1.0) / (spp + stt + 1.0)
    return np.float32(per_img.mean())


_NC_CACHE = {}


def kernel(seg_feat, conv_weight, mask, ind, target):
    if "nc" not in _NC_CACHE:
        _NC_CACHE["nc"] = build_nc()
    nc = _NC_CACHE["nc"]
    in_maps, tt_host = prep_inputs(seg_feat, conv_weight, mask, ind, target)
    res = run_bass_kernel_spmd(nc, in_maps, list(range(N_CORES)))
    return finish([res.results[b]["red"] for b in range(B)], tt_host)
